# revision 15
# baseline (speedup 1.0000x reference)
"""Multi-head self-attention (B=2, S=2048, D=1024, H=16, depth=64) on 8
Trainium2 NeuronCores.

Sharding: core c handles batch c//4 and the 4 heads [4*(c%4), 4*(c%4)+4).
Data-parallel on batch, tensor-parallel on heads: each core computes its
heads' Q/K/V projections (column-sharded weights), the full S x S softmax
attention for those heads (written out as the `attn` output), and a partial
output projection (row-sharded wo) that the host sums per batch.

Per-core kernel (matmuls in float32r: fp32 bits, single-pass reduced
precision on the PE at ~1 cycle/row for c=128; softmax exp in fp32 on the
scalar engine):
  A) transpose x -> xT [D, S] via PE transposes (the PE contracts along the
     partition dim, so both matmul operands need D on partitions).
  B) projections. qT/kT are stored per head as [128, S] with the head's 64
     depth rows on partitions 0-63 and ZEROS on 64-127: c=64 matmuls run
     2x slower than c=128 on the fp32 path, so we pad the contraction with
     zeros instead. V [S, 4*64] is stored with a ones column per head.
  C) per head: logits^T tiles -> exp -> E^T; ctx^T accumulation with the
     stationary [V | 1] so PSUM row 64 accumulates the softmax denominators
     r for free. 1/r is broadcast across partitions with a c=1 matmul to
     normalize ctx^T, and transposed to [128, 16] with tiny c=1 matmuls for
     phase D's per-row scaling.
  D) per head: natural-layout logits -> exp -> P = E * (1/r) -> DMA to
     attn. Interleaved per head so the 64 MiB of attn stores spread across
     the whole kernel.
  E) output projection from the normalized ctx^T chunks (after C, before
     the last head's D).
"""

import numpy as np

import concourse.bass as bass
import concourse.mybir as mybir
import concourse.tile as tile
from concourse.bass_utils import run_bass_kernel_spmd
from concourse.vector_clock import ScopedClock

F32 = mybir.dt.float32
F32R = mybir.dt.float32r
EXP = mybir.ActivationFunctionType.Exp

S = 2048
D = 1024
HL = 4          # heads per core
DEP = 64        # head depth
NCORES = 8
SCALE = 0.125   # 1/sqrt(DEP)

NS = S // 128   # 16 chunks of 128
NCH = D // 128  # 8 contraction chunks
NT = S // 512   # 4 tiles of 512

# ---------------------------------------------------------------------------
# walrus in this toolchain rejects >1 sync-wait per instruction; split extras
# onto NOPs inserted before the instruction on the same engine.
_ctr = [0]


def _split_sync_waits(nc, max_waits=1):
    for f in nc.m.functions:
        for bb in f.blocks:
            new = []
            changed = False
            for inst in bb.instructions:
                si = inst.sync_info
                if si is not None and len(si.on_wait) > max_waits:
                    waits = list(si.on_wait)
                    for w in waits[:-max_waits]:
                        _ctr[0] += 1
                        nop = mybir.InstNoOp(
                            name=f"I-wsplit-{_ctr[0]}", ins=[], outs=[]
                        )
                        nop.engine = inst.engine
                        nop.sync_info = mybir.SyncInfo(on_wait=[w], on_update=[])
                        new.append(nop)
                    si.on_wait = waits[-max_waits:]
                    changed = True
                new.append(inst)
            if changed:
                bb.instructions = new


class _TileContextCompat(tile.TileContext):
    def _drain_and_barrier(self, tick_clock, wait_clock):
        drain_inst = self.nc.sync.drain()
        wait_clock.add_sem_waits(
            drain_inst.ins, ScopedClock({None: tick_clock.global_clock})
        )
        self.nc.all_engine_barrier()
        assert self.sems is not None
        popped = self.nc._tile_sem_poison_stack.pop()
        assert popped is self._sem_poison
        self.nc.clear_and_free_semaphores(list(self.sems.allocated().values()))
        self.nc.all_engine_barrier()

    def __exit__(self, *args):
        ret = super().__exit__(*args)
        if args[0] is None:
            _split_sync_waits(self.nc)
        return ret


# ---------------------------------------------------------------------------
def build_attention_nc():
    nc = bass.Bass("TRN2", target_bir_lowering=False, debug=False, num_devices=1)

    x_d = nc.dram_tensor("x", [S, D], F32, kind="ExternalInput").ap()
    wq_d = nc.dram_tensor("wq", [D, HL * DEP], F32, kind="ExternalInput").ap()
    wk_d = nc.dram_tensor("wk", [D, HL * DEP], F32, kind="ExternalInput").ap()
    wv_d = nc.dram_tensor("wv", [D, HL * DEP], F32, kind="ExternalInput").ap()
    wo_d = nc.dram_tensor("wo", [HL * DEP, D], F32, kind="ExternalInput").ap()
    id_d = nc.dram_tensor("ident", [128, 128], F32, kind="ExternalInput").ap()

    attn_d = nc.dram_tensor("attn", [HL, S, S], F32, kind="ExternalOutput").ap()
    outp_d = nc.dram_tensor("outp", [S, D], F32, kind="ExternalOutput").ap()

    with _TileContextCompat(nc) as tc:
        with (
            tc.tile_pool(name="persist", bufs=1) as pp,
            tc.tile_pool(name="psmm", bufs=3, space="PSUM") as psmm,
            tc.tile_pool(name="psctx", bufs=1, space="PSUM") as psctx,
        ):
            # persistent SBUF tensors. qTz/kTz: per-head [128, S], rows 64-127
            # zeroed so logits matmuls run with c=128.
            qTz = [pp.tile([128, S], F32R, name=f"qTz{h}", tag=f"qTz{h}") for h in range(HL)]
            kTz = [pp.tile([128, S], F32R, name=f"kTz{h}", tag=f"kTz{h}") for h in range(HL)]
            vsb = pp.tile([128, NS * HL * 65], F32R, tag="vsb")
            vsb3 = vsb[:].rearrange("p (s h e) -> p s h e", s=NS, h=HL)
            wo_sb = [pp.tile([128, D], F32R, name=f"wo{cc}", tag=f"wo{cc}") for cc in range(2)]
            ctxT = [pp.tile([128, S], F32R, name=f"ctxT{m}", tag=f"ctxT{m}") for m in range(2)]
            ones64 = pp.tile([128, 64], F32, tag="ones64")
            id_t = pp.tile([128, 128], F32, tag="ident")
            nc.gpsimd.dma_start(id_t[:], id_d[:])

            nc.vector.memset(ones64[:], 1.0)
            ones_r = pp.tile([1, 64], F32R, tag="ones_r")
            nc.vector.tensor_copy(ones_r[:], ones64[0:1, :])
            id_r = pp.tile([128, 128], F32R, tag="id_r")
            nc.vector.tensor_copy(id_r[:], id_t[:])
            # dense dummy matmul burst to flip the PE clock gate to 8/8
            # before real work starts (~3.4us of sustained PE activity)
            pwarm = psmm.tile([128, 1024], F32, tag="mm")
            for i in range(36):
                nc.tensor.matmul(
                    pwarm[:, 0:128], id_r[:], id_r[:], start=True, stop=True
                )
            nc.vector.tensor_copy(
                vsb3[:, :, :, 64:65].rearrange("p s h e -> p s (h e)"),
                ones64[:].rearrange("p (s h) -> p s h", s=NS, h=HL),
            )

            # ---------------- phase A + B ----------------
            with (
                tc.tile_pool(name="phAB", bufs=1) as ab,
                tc.tile_pool(name="xin", bufs=2) as xin,
            ):
                # zero rows 64-127 of qTz/kTz
                zrow = ab.tile([64, 1024], F32, tag="zrow")
                nc.vector.memset(zrow[:], 0.0)
                for h in range(HL):
                    for half in range(2):
                        sl = slice(1024 * half, 1024 * (half + 1))
                        nc.vector.tensor_copy(qTz[h][64:128, sl], zrow[:])
                        nc.vector.tensor_copy(kTz[h][64:128, sl], zrow[:])

                xT = ab.tile([128, NCH * S], F32R, tag="xT")
                xT3 = xT[:].rearrange("p (c s) -> p c s", c=NCH)
                w_sb = {}
                for nm, src in (("wq", wq_d), ("wk", wk_d), ("wv", wv_d)):
                    t = ab.tile([128, NCH * HL * DEP], F32R, name=f"w_{nm}", tag=f"w_{nm}")
                    t3 = t[:].rearrange("p (c n) -> p c n", c=NCH)
                    for cc in range(NCH):
                        nc.gpsimd.dma_start(
                            t3[:, cc, :], src[128 * cc : 128 * (cc + 1), :]
                        )
                    w_sb[nm] = t3
                for cc in range(2):
                    nc.gpsimd.dma_start(
                        wo_sb[cc][:], wo_d[128 * cc : 128 * (cc + 1), :]
                    )

                # A: load + transpose x
                for s in range(NS):
                    xt = xin.tile([128, D], F32, tag="x_in")
                    nc.sync.dma_start(xt[:], x_d[128 * s : 128 * (s + 1), :])
                    pt = psmm.tile([128, 1024], F32, tag="mm")
                    for c in range(NCH):
                        nc.tensor.transpose(
                            pt[:, 128 * c : 128 * (c + 1)],
                            xt[:, 128 * c : 128 * (c + 1)],
                            id_t[:],
                        )
                    nc.vector.tensor_copy(
                        xT3[:, :, 128 * s : 128 * (s + 1)],
                        pt[:].rearrange("p (c s) -> p c s", c=NCH),
                    )

                # B: projections. psum tile holds 2 heads [128, 512]; evict
                # each head's 64 rows into its padded tensor.
                for nm, dst in (("wq", qTz), ("wk", kTz)):
                    for m in range(2):
                        pta = psmm.tile([128, 1024], F32, tag="mm")
                        ptb = psmm.tile([128, 1024], F32, tag="mm")
                        halves = [
                            pta[:, 0:512], pta[:, 512:1024],
                            ptb[:, 0:512], ptb[:, 512:1024],
                        ]
                        for c in range(NCH):
                            for t in range(NT):
                                nc.tensor.matmul(
                                    halves[t],
                                    w_sb[nm][:, c, 128 * m : 128 * (m + 1)],
                                    xT3[:, c, 512 * t : 512 * (t + 1)],
                                    start=(c == 0),
                                    stop=(c == NCH - 1),
                                )
                        for t in range(NT):
                            for hh in range(2):
                                nc.vector.tensor_copy(
                                    dst[2 * m + hh][0:64, 512 * t : 512 * (t + 1)],
                                    halves[t][64 * hh : 64 * (hh + 1), :],
                                )
                for s in range(NS):
                    pt = psmm.tile([128, 1024], F32, tag="mm")
                    for c in range(NCH):
                        nc.tensor.matmul(
                            pt[:, 0:256],
                            xT3[:, c, 128 * s : 128 * (s + 1)],
                            w_sb["wv"][:, c, :],
                            start=(c == 0),
                            stop=(c == NCH - 1),
                        )
                    nc.vector.tensor_copy(
                        vsb3[:, s, :, 0:64],
                        pt[:, 0:256].rearrange("p (h e) -> p h e", h=HL),
                    )

            # ---------------- phases C / D / E ----------------
            with (
                tc.tile_pool(name="phCD", bufs=2) as cd,
                tc.tile_pool(name="phC3", bufs=3) as cd3,
                tc.tile_pool(name="cds", bufs=1) as cds,
                tc.tile_pool(name="phE", bufs=2) as ep,
            ):

                def phase_C(h):
                    m, off = h // 2, 64 * (h % 2)
                    r_r = cds.tile([1, S], F32R, tag="r_r")
                    ctmp = cds.tile([64, S], F32, tag="ctmp")
                    for qh in range(2):
                        qsl = slice(1024 * qh, 1024 * (qh + 1))
                        pch = psctx.tile([65, 1024], F32, tag="ctxh")
                        for j in range(NS):
                            pl = psmm.tile([128, 1024], F32, tag="mm")
                            for u in range(2):
                                nc.tensor.matmul(
                                    pl[:, 512 * u : 512 * (u + 1)],
                                    kTz[h][:, 128 * j : 128 * (j + 1)],
                                    qTz[h][
                                        :,
                                        1024 * qh + 512 * u : 1024 * qh + 512 * (u + 1),
                                    ],
                                    start=True,
                                    stop=True,
                                )
                            et = cd3.tile([128, 1024], F32R, tag="eT")
                            nc.scalar.activation(et[:], pl[:], EXP, scale=SCALE)
                            vl = vsb3[:, j, h, :]  # [128, 65]
                            for u in range(2):
                                nc.tensor.matmul(
                                    pch[:, 512 * u : 512 * (u + 1)],
                                    vl,
                                    et[:, 512 * u : 512 * (u + 1)],
                                    start=(j == 0),
                                    stop=(j == NS - 1),
                                )
                        nc.vector.tensor_copy(r_r[:, qsl], pch[64:65, :])
                        nc.vector.tensor_copy(ctmp[:, qsl], pch[0:64, :])
                    # transpose r to [128, 16] via c=1 outer products (16
                    # identical columns each; keep column 0); reciprocal in
                    # the partition layout (fast: FD=16 vs 13us at FD=2048).
                    pr = psmm.tile([128, 1024], F32, tag="mm")
                    for t in range(NS):
                        nc.tensor.matmul(
                            pr[:, 16 * t : 16 * (t + 1)],
                            r_r[:, 128 * t : 128 * (t + 1)],
                            ones_r[:, 0:16],
                            start=True,
                            stop=True,
                        )
                    rinvT = cds.tile([128, NS], F32, tag="rinvT")
                    nc.vector.reciprocal(
                        rinvT[:],
                        pr[:, 0 : 16 * NS].rearrange("p (t e) -> p t e", t=NS)[:, :, 0],
                    )
                    rT_r = cds.tile([128, NS], F32R, tag="rT_r")
                    nc.vector.tensor_copy(rT_r[:], rinvT[:])
                    # broadcast 1/r across 64 partitions: lhsT is rT_r's
                    # column t broadcast (stride 0) over 64 weight columns,
                    # contracted against the identity -> pb[p, q] = rinvT[q,t]
                    for qh in range(2):
                        qsl = slice(1024 * qh, 1024 * (qh + 1))
                        pb = psctx.tile([65, 1024], F32, tag="ctxh")
                        for tt in range(8):
                            t = 8 * qh + tt
                            nc.tensor.matmul(
                                pb[0:64, 128 * tt : 128 * (tt + 1)],
                                rT_r[:, t : t + 1].broadcast_to((128, 64)),
                                id_r[:],
                                start=True,
                                stop=True,
                            )
                        nc.vector.tensor_mul(
                            ctxT[m][off : off + 64, qsl], ctmp[:, qsl], pb[0:64, :]
                        )
                    return rinvT

                def phase_D(h, rinvT, interleave_E=False):
                    for t in range(NS):
                        if interleave_E:
                            phase_E_tile(t)
                        e_t = cd.tile([128, S], F32, tag="e_t")
                        for half in range(2):
                            pn = psmm.tile([128, 1024], F32, tag="mm")
                            for u in range(2):
                                nc.tensor.matmul(
                                    pn[:, 512 * u : 512 * (u + 1)],
                                    qTz[h][:, 128 * t : 128 * (t + 1)],
                                    kTz[h][
                                        :,
                                        1024 * half + 512 * u : 1024 * half + 512 * (u + 1),
                                    ],
                                    start=True,
                                    stop=True,
                                )
                            nc.scalar.activation(
                                e_t[:, 1024 * half : 1024 * (half + 1)],
                                pn[:],
                                EXP,
                                scale=SCALE,
                            )
                        p_t = cd.tile([128, S], F32, tag="p_t")
                        nc.vector.tensor_scalar_mul(
                            p_t[:], e_t[:], rinvT[:, t : t + 1]
                        )
                        nc.sync.dma_start(
                            attn_d[h, 128 * t : 128 * (t + 1), :], p_t[:]
                        )

                def phase_E_tile(t):
                        po = psmm.tile([128, 1024], F32, tag="mm")
                        for nh in range(2):
                            for cc in range(2):
                                nc.tensor.matmul(
                                    po[:, 512 * nh : 512 * (nh + 1)],
                                    ctxT[cc][:, 128 * t : 128 * (t + 1)],
                                    wo_sb[cc][:, 512 * nh : 512 * (nh + 1)],
                                    start=(cc == 0),
                                    stop=(cc == 1),
                                )
                        osb = ep.tile([128, D], F32, tag="osb")
                        nc.vector.tensor_copy(osb[:], po[:])
                        nc.sync.dma_start(
                            outp_d[128 * t : 128 * (t + 1), :], osb[:]
                        )

                for h in range(HL):
                    rinvT = phase_C(h)
                    phase_D(h, rinvT, interleave_E=(h == HL - 1))

    return nc


_NC_CACHE = None


def kernel(inputs, wq, bq, wk, bk, wv, bv, wo, bo):
    global _NC_CACHE
    inputs = np.asarray(inputs, np.float32)
    wq = np.asarray(wq, np.float32)
    wk = np.asarray(wk, np.float32)
    wv = np.asarray(wv, np.float32)
    wo = np.asarray(wo, np.float32)
    bo = np.asarray(bo, np.float32)
    for b_ in (bq, bk, bv):
        assert not np.any(np.asarray(b_)), "nonzero qkv biases not supported"

    B = inputs.shape[0]
    H = 16
    ident = np.eye(128, dtype=np.float32)

    in_maps = []
    for c in range(NCORES):
        b, g = c // 4, c % 4
        cols = slice(HL * DEP * g, HL * DEP * (g + 1))
        in_maps.append(
            {
                "x": np.ascontiguousarray(inputs[b]),
                "wq": np.ascontiguousarray(wq[:, cols]),
                "wk": np.ascontiguousarray(wk[:, cols]),
                "wv": np.ascontiguousarray(wv[:, cols]),
                "wo": np.ascontiguousarray(wo[cols, :]),
                "ident": ident,
            }
        )

    if _NC_CACHE is None:
        _NC_CACHE = build_attention_nc()
    res = run_bass_kernel_spmd(
        _NC_CACHE, in_maps, core_ids=list(range(NCORES)), trace=False
    )

    attn = np.empty((B, H, S, S), np.float32)
    out = np.zeros((B, S, D), np.float32)
    for c in range(NCORES):
        b, g = c // 4, c % 4
        attn[b, HL * g : HL * (g + 1)] = res.results[c]["attn"]
        out[b] += res.results[c]["outp"]
    out += bo[None, None, :]
    return out, attn


# revision 16
# speedup vs baseline: 1.0188x; 1.0188x over previous
"""Multi-head self-attention (B=2, S=2048, D=1024, H=16, depth=64) on 8
Trainium2 NeuronCores.

Sharding: core c handles batch c//4 and the 4 heads [4*(c%4), 4*(c%4)+4).
Data-parallel on batch, tensor-parallel on heads: each core computes its
heads' Q/K/V projections (column-sharded weights), the full S x S softmax
attention for those heads (written out as the `attn` output), and a partial
output projection (row-sharded wo) that the host sums per batch.

Per-core kernel (matmuls in float32r: fp32 bits, single-pass reduced
precision on the PE at ~1 cycle/row for c=128; softmax exp in fp32 on the
scalar engine):
  A) transpose x -> xT [D, S] via PE transposes (the PE contracts along the
     partition dim, so both matmul operands need D on partitions).
  B) projections. qT/kT are stored per head as [128, S] with the head's 64
     depth rows on partitions 0-63 and ZEROS on 64-127: c=64 matmuls run
     2x slower than c=128 on the fp32 path, so we pad the contraction with
     zeros instead. V [S, 4*64] is stored with a ones column per head.
  C) per head: logits^T tiles -> exp -> E^T; ctx^T accumulation with the
     stationary [V | 1] so PSUM row 64 accumulates the softmax denominators
     r for free. 1/r is broadcast across partitions with a c=1 matmul to
     normalize ctx^T, and transposed to [128, 16] with tiny c=1 matmuls for
     phase D's per-row scaling.
  D) per head: natural-layout logits -> exp -> P = E * (1/r) -> DMA to
     attn. Interleaved per head so the 64 MiB of attn stores spread across
     the whole kernel.
  E) output projection from the normalized ctx^T chunks (after C, before
     the last head's D).
"""

import numpy as np

import concourse.bass as bass
import concourse.mybir as mybir
import concourse.tile as tile
import concourse.bass_utils as _bass_utils
from concourse.bass_utils import run_bass_kernel_spmd
from concourse.vector_clock import ScopedClock

# let walrus elide redundant LDWEIGHTS (off by default in this toolchain)
if not getattr(_bass_utils, "_ldw_opt_patched", False):
    _orig_run_command = _bass_utils.run_command

    def _run_command_ldw(argv, **kw):
        argv = [
            a.replace("--enable-ldw-opt=false", "--enable-ldw-opt=true")
            if isinstance(a, str)
            else a
            for a in argv
        ]
        return _orig_run_command(argv, **kw)

    _bass_utils.run_command = _run_command_ldw
    _bass_utils._ldw_opt_patched = True

F32 = mybir.dt.float32
F32R = mybir.dt.float32r
EXP = mybir.ActivationFunctionType.Exp

S = 2048
D = 1024
HL = 4          # heads per core
DEP = 64        # head depth
NCORES = 8
SCALE = 0.125   # 1/sqrt(DEP)

NS = S // 128   # 16 chunks of 128
NCH = D // 128  # 8 contraction chunks
NT = S // 512   # 4 tiles of 512

# ---------------------------------------------------------------------------
# walrus in this toolchain rejects >1 sync-wait per instruction; split extras
# onto NOPs inserted before the instruction on the same engine.
_ctr = [0]


def _split_sync_waits(nc, max_waits=1):
    for f in nc.m.functions:
        for bb in f.blocks:
            new = []
            changed = False
            for inst in bb.instructions:
                si = inst.sync_info
                if si is not None and len(si.on_wait) > max_waits:
                    waits = list(si.on_wait)
                    for w in waits[:-max_waits]:
                        _ctr[0] += 1
                        nop = mybir.InstNoOp(
                            name=f"I-wsplit-{_ctr[0]}", ins=[], outs=[]
                        )
                        nop.engine = inst.engine
                        nop.sync_info = mybir.SyncInfo(on_wait=[w], on_update=[])
                        new.append(nop)
                    si.on_wait = waits[-max_waits:]
                    changed = True
                new.append(inst)
            if changed:
                bb.instructions = new


class _TileContextCompat(tile.TileContext):
    def _drain_and_barrier(self, tick_clock, wait_clock):
        drain_inst = self.nc.sync.drain()
        wait_clock.add_sem_waits(
            drain_inst.ins, ScopedClock({None: tick_clock.global_clock})
        )
        self.nc.all_engine_barrier()
        assert self.sems is not None
        popped = self.nc._tile_sem_poison_stack.pop()
        assert popped is self._sem_poison
        self.nc.clear_and_free_semaphores(list(self.sems.allocated().values()))
        self.nc.all_engine_barrier()

    def __exit__(self, *args):
        ret = super().__exit__(*args)
        if args[0] is None:
            _split_sync_waits(self.nc)
        return ret


# ---------------------------------------------------------------------------
def build_attention_nc():
    nc = bass.Bass("TRN2", target_bir_lowering=False, debug=False, num_devices=1)

    x_d = nc.dram_tensor("x", [S, D], F32, kind="ExternalInput").ap()
    wq_d = nc.dram_tensor("wq", [D, HL * DEP], F32, kind="ExternalInput").ap()
    wk_d = nc.dram_tensor("wk", [D, HL * DEP], F32, kind="ExternalInput").ap()
    wv_d = nc.dram_tensor("wv", [D, HL * DEP], F32, kind="ExternalInput").ap()
    wo_d = nc.dram_tensor("wo", [HL * DEP, D], F32, kind="ExternalInput").ap()
    id_d = nc.dram_tensor("ident", [128, 128], F32, kind="ExternalInput").ap()

    attn_d = nc.dram_tensor("attn", [HL, S, S], F32, kind="ExternalOutput").ap()
    outp_d = nc.dram_tensor("outp", [S, D], F32, kind="ExternalOutput").ap()

    with _TileContextCompat(nc) as tc:
        with (
            tc.tile_pool(name="persist", bufs=1) as pp,
            tc.tile_pool(name="psmm", bufs=3, space="PSUM") as psmm,
            tc.tile_pool(name="psctx", bufs=1, space="PSUM") as psctx,
        ):
            # persistent SBUF tensors. qTz/kTz: per-head [128, S], rows 64-127
            # zeroed so logits matmuls run with c=128.
            qTz = [pp.tile([128, S], F32R, name=f"qTz{h}", tag=f"qTz{h}") for h in range(HL)]
            kTz = [pp.tile([128, S], F32R, name=f"kTz{h}", tag=f"kTz{h}") for h in range(HL)]
            vsb = pp.tile([128, NS * HL * 65], F32R, tag="vsb")
            vsb3 = vsb[:].rearrange("p (s h e) -> p s h e", s=NS, h=HL)
            wo_sb = [pp.tile([128, D], F32R, name=f"wo{cc}", tag=f"wo{cc}") for cc in range(2)]
            ctxT = [pp.tile([128, S], F32R, name=f"ctxT{m}", tag=f"ctxT{m}") for m in range(2)]
            ones64 = pp.tile([128, 64], F32, tag="ones64")
            id_t = pp.tile([128, 128], F32, tag="ident")
            nc.gpsimd.dma_start(id_t[:], id_d[:])

            nc.vector.memset(ones64[:], 1.0)
            ones_r = pp.tile([1, 64], F32R, tag="ones_r")
            nc.vector.tensor_copy(ones_r[:], ones64[0:1, :])
            id_r = pp.tile([128, 128], F32R, tag="id_r")
            nc.vector.tensor_copy(id_r[:], id_t[:])
            # dense dummy matmul burst to flip the PE clock gate to 8/8
            # before real work starts (~3.4us of sustained PE activity)
            pwarm = psmm.tile([128, 1024], F32, tag="mm")
            for i in range(36):
                nc.tensor.matmul(
                    pwarm[:, 0:128], id_r[:], id_r[:], start=True, stop=True
                )
            nc.vector.tensor_copy(
                vsb3[:, :, :, 64:65].rearrange("p s h e -> p s (h e)"),
                ones64[:].rearrange("p (s h) -> p s h", s=NS, h=HL),
            )

            # ---------------- phase A + B ----------------
            with (
                tc.tile_pool(name="phAB", bufs=1) as ab,
                tc.tile_pool(name="xin", bufs=2) as xin,
            ):
                # zero rows 64-127 of qTz/kTz
                zrow = ab.tile([64, 1024], F32, tag="zrow")
                nc.vector.memset(zrow[:], 0.0)
                for h in range(HL):
                    for half in range(2):
                        sl = slice(1024 * half, 1024 * (half + 1))
                        nc.vector.tensor_copy(qTz[h][64:128, sl], zrow[:])
                        nc.vector.tensor_copy(kTz[h][64:128, sl], zrow[:])

                xT = ab.tile([128, NCH * S], F32R, tag="xT")
                xT3 = xT[:].rearrange("p (c s) -> p c s", c=NCH)
                w_sb = {}
                for nm, src in (("wq", wq_d), ("wk", wk_d), ("wv", wv_d)):
                    t = ab.tile([128, NCH * HL * DEP], F32R, name=f"w_{nm}", tag=f"w_{nm}")
                    t3 = t[:].rearrange("p (c n) -> p c n", c=NCH)
                    for cc in range(NCH):
                        nc.gpsimd.dma_start(
                            t3[:, cc, :], src[128 * cc : 128 * (cc + 1), :]
                        )
                    w_sb[nm] = t3
                for cc in range(2):
                    nc.gpsimd.dma_start(
                        wo_sb[cc][:], wo_d[128 * cc : 128 * (cc + 1), :]
                    )

                # A: load + transpose x
                for s in range(NS):
                    xt = xin.tile([128, D], F32, tag="x_in")
                    nc.sync.dma_start(xt[:], x_d[128 * s : 128 * (s + 1), :])
                    pt = psmm.tile([128, 1024], F32, tag="mm")
                    for c in range(NCH):
                        nc.tensor.transpose(
                            pt[:, 128 * c : 128 * (c + 1)],
                            xt[:, 128 * c : 128 * (c + 1)],
                            id_t[:],
                        )
                    nc.vector.tensor_copy(
                        xT3[:, :, 128 * s : 128 * (s + 1)],
                        pt[:].rearrange("p (c s) -> p c s", c=NCH),
                    )

                # B: projections. psum tile holds 2 heads [128, 512]; evict
                # each head's 64 rows into its padded tensor.
                for nm, dst in (("wq", qTz), ("wk", kTz)):
                    for m in range(2):
                        pta = psmm.tile([128, 1024], F32, tag="mm")
                        ptb = psmm.tile([128, 1024], F32, tag="mm")
                        halves = [
                            pta[:, 0:512], pta[:, 512:1024],
                            ptb[:, 0:512], ptb[:, 512:1024],
                        ]
                        for c in range(NCH):
                            for t in range(NT):
                                nc.tensor.matmul(
                                    halves[t],
                                    w_sb[nm][:, c, 128 * m : 128 * (m + 1)],
                                    xT3[:, c, 512 * t : 512 * (t + 1)],
                                    start=(c == 0),
                                    stop=(c == NCH - 1),
                                )
                        for t in range(NT):
                            for hh in range(2):
                                nc.vector.tensor_copy(
                                    dst[2 * m + hh][0:64, 512 * t : 512 * (t + 1)],
                                    halves[t][64 * hh : 64 * (hh + 1), :],
                                )
                for s in range(NS):
                    pt = psmm.tile([128, 1024], F32, tag="mm")
                    for c in range(NCH):
                        nc.tensor.matmul(
                            pt[:, 0:256],
                            xT3[:, c, 128 * s : 128 * (s + 1)],
                            w_sb["wv"][:, c, :],
                            start=(c == 0),
                            stop=(c == NCH - 1),
                        )
                    nc.vector.tensor_copy(
                        vsb3[:, s, :, 0:64],
                        pt[:, 0:256].rearrange("p (h e) -> p h e", h=HL),
                    )

            # ---------------- phases C / D / E ----------------
            with (
                tc.tile_pool(name="phCD", bufs=2) as cd,
                tc.tile_pool(name="phC3", bufs=3) as cd3,
                tc.tile_pool(name="cds", bufs=1) as cds,
                tc.tile_pool(name="phE", bufs=2) as ep,
            ):

                def phase_C(h):
                    m, off = h // 2, 64 * (h % 2)
                    r_r = cds.tile([1, S], F32R, tag="r_r")
                    ctmp = cds.tile([64, S], F32, tag="ctmp")
                    for qh in range(2):
                        qsl = slice(1024 * qh, 1024 * (qh + 1))
                        pch = psctx.tile([65, 1024], F32, tag="ctxh")
                        for j in range(NS):
                            pl = psmm.tile([128, 1024], F32, tag="mm")
                            for u in range(2):
                                nc.tensor.matmul(
                                    pl[:, 512 * u : 512 * (u + 1)],
                                    kTz[h][:, 128 * j : 128 * (j + 1)],
                                    qTz[h][
                                        :,
                                        1024 * qh + 512 * u : 1024 * qh + 512 * (u + 1),
                                    ],
                                    start=True,
                                    stop=True,
                                )
                            et = cd3.tile([128, 1024], F32R, tag="eT")
                            nc.scalar.activation(et[:], pl[:], EXP, scale=SCALE)
                            vl = vsb3[:, j, h, :]  # [128, 65]
                            for u in range(2):
                                nc.tensor.matmul(
                                    pch[:, 512 * u : 512 * (u + 1)],
                                    vl,
                                    et[:, 512 * u : 512 * (u + 1)],
                                    start=(j == 0),
                                    stop=(j == NS - 1),
                                )
                        nc.vector.tensor_copy(r_r[:, qsl], pch[64:65, :])
                        nc.vector.tensor_copy(ctmp[:, qsl], pch[0:64, :])
                    # transpose r to [128, 16] via c=1 outer products (16
                    # identical columns each; keep column 0); reciprocal in
                    # the partition layout (fast: FD=16 vs 13us at FD=2048).
                    pr = psmm.tile([128, 1024], F32, tag="mm")
                    for t in range(NS):
                        nc.tensor.matmul(
                            pr[:, 16 * t : 16 * (t + 1)],
                            r_r[:, 128 * t : 128 * (t + 1)],
                            ones_r[:, 0:16],
                            start=True,
                            stop=True,
                        )
                    rinvT = cds.tile([128, NS], F32, tag="rinvT")
                    nc.vector.reciprocal(
                        rinvT[:],
                        pr[:, 0 : 16 * NS].rearrange("p (t e) -> p t e", t=NS)[:, :, 0],
                    )
                    rT_r = cds.tile([128, NS], F32R, tag="rT_r")
                    nc.vector.tensor_copy(rT_r[:], rinvT[:])
                    # broadcast 1/r across 64 partitions: lhsT is rT_r's
                    # column t broadcast (stride 0) over 64 weight columns,
                    # contracted against the identity -> pb[p, q] = rinvT[q,t]
                    for qh in range(2):
                        qsl = slice(1024 * qh, 1024 * (qh + 1))
                        pb = psctx.tile([65, 1024], F32, tag="ctxh")
                        for tt in range(8):
                            t = 8 * qh + tt
                            nc.tensor.matmul(
                                pb[0:64, 128 * tt : 128 * (tt + 1)],
                                rT_r[:, t : t + 1].broadcast_to((128, 64)),
                                id_r[:],
                                start=True,
                                stop=True,
                            )
                        nc.vector.tensor_mul(
                            ctxT[m][off : off + 64, qsl], ctmp[:, qsl], pb[0:64, :]
                        )
                    return rinvT

                def phase_D(h, rinvT):
                    for t in range(NS):
                        e_t = cd.tile([128, S], F32, tag="e_t")
                        for half in range(2):
                            pn = psmm.tile([128, 1024], F32, tag="mm")
                            for u in range(2):
                                nc.tensor.matmul(
                                    pn[:, 512 * u : 512 * (u + 1)],
                                    qTz[h][:, 128 * t : 128 * (t + 1)],
                                    kTz[h][
                                        :,
                                        1024 * half + 512 * u : 1024 * half + 512 * (u + 1),
                                    ],
                                    start=True,
                                    stop=True,
                                )
                            nc.scalar.activation(
                                e_t[:, 1024 * half : 1024 * (half + 1)],
                                pn[:],
                                EXP,
                                scale=SCALE,
                            )
                        p_t = cd.tile([128, S], F32, tag="p_t")
                        nc.vector.tensor_scalar_mul(
                            p_t[:], e_t[:], rinvT[:, t : t + 1]
                        )
                        nc.sync.dma_start(
                            attn_d[h, 128 * t : 128 * (t + 1), :], p_t[:]
                        )

                def phase_E():
                    for t in range(NS):
                        po = psmm.tile([128, 1024], F32, tag="mm")
                        for nh in range(2):
                            for cc in range(2):
                                nc.tensor.matmul(
                                    po[:, 512 * nh : 512 * (nh + 1)],
                                    ctxT[cc][:, 128 * t : 128 * (t + 1)],
                                    wo_sb[cc][:, 512 * nh : 512 * (nh + 1)],
                                    start=(cc == 0),
                                    stop=(cc == 1),
                                )
                        osb = ep.tile([128, D], F32, tag="osb")
                        nc.vector.tensor_copy(osb[:], po[:])
                        nc.sync.dma_start(
                            outp_d[128 * t : 128 * (t + 1), :], osb[:]
                        )

                for h in range(HL):
                    rinvT = phase_C(h)
                    phase_D(h, rinvT)
                phase_E()

    return nc


_NC_CACHE = None


def kernel(inputs, wq, bq, wk, bk, wv, bv, wo, bo):
    global _NC_CACHE
    inputs = np.asarray(inputs, np.float32)
    wq = np.asarray(wq, np.float32)
    wk = np.asarray(wk, np.float32)
    wv = np.asarray(wv, np.float32)
    wo = np.asarray(wo, np.float32)
    bo = np.asarray(bo, np.float32)
    for b_ in (bq, bk, bv):
        assert not np.any(np.asarray(b_)), "nonzero qkv biases not supported"

    B = inputs.shape[0]
    H = 16
    ident = np.eye(128, dtype=np.float32)

    in_maps = []
    for c in range(NCORES):
        b, g = c // 4, c % 4
        cols = slice(HL * DEP * g, HL * DEP * (g + 1))
        in_maps.append(
            {
                "x": np.ascontiguousarray(inputs[b]),
                "wq": np.ascontiguousarray(wq[:, cols]),
                "wk": np.ascontiguousarray(wk[:, cols]),
                "wv": np.ascontiguousarray(wv[:, cols]),
                "wo": np.ascontiguousarray(wo[cols, :]),
                "ident": ident,
            }
        )

    if _NC_CACHE is None:
        _NC_CACHE = build_attention_nc()
    res = run_bass_kernel_spmd(
        _NC_CACHE, in_maps, core_ids=list(range(NCORES)), trace=False
    )

    attn = np.empty((B, H, S, S), np.float32)
    out = np.zeros((B, S, D), np.float32)
    for c in range(NCORES):
        b, g = c // 4, c % 4
        attn[b, HL * g : HL * (g + 1)] = res.results[c]["attn"]
        out[b] += res.results[c]["outp"]
    out += bo[None, None, :]
    return out, attn
